# revision 8
# baseline (speedup 1.0000x reference)
"""Trainium2 Bass kernel for a 2-layer GATv2 encoder (nn_CG_GNN_Encoder).

kernel(**inputs) takes full inputs (x [20000,512] f32, edge_index [2,320000]
int64, weights) and returns the full [20000, 512] f32 output, across 8 cores.

v2 design (per core, dst-node sharded):
  - Host: balance dst nodes into 8 cores x 20 blocks x 125 nodes; per-block
    edge lists padded to e_blk; UNWEIGHTED one-hot scatter matrices built on
    host (pad edges get all-zero columns -> no masks needed); |att|*(2/3)
    folded into Wl/Wr columns; 4 augmented weight columns compute the
    separable logit term A[n,h] = sum_c sigma_c xl'[n,c] on the fly.
  - Phase A per layer: x tiles -> PE transpose -> matmuls -> xl/xr [.,516]
    (512 features + 4 aug cols), bias added during PSUM evacuation.
  - AllGather xl across 8 cores; xr stays local.
  - Edge phase per block: 2 batched indirect gathers (xl[src], xr[dst]),
    t = xl+xr (one DVE add), logits = t_aug + (sum_pos|t| - sum_neg|t|)
    via 8 abs-reduces, p = exp(0.6*lg) written straight into the value
    tile's appended p-columns, xa = xl*p via broadcast mult, then one
    one-hot matmul per (s, head-pair) accumulating values+denominators
    in PSUM; normalize, ELU between layers.
"""

import numpy as np
from ml_dtypes import bfloat16

import concourse.bacc as bacc
import concourse.bass as bass
import concourse.mybir as mybir
import concourse.tile as tile
from concourse.bass_utils import run_bass_kernel_spmd

F32 = mybir.dt.float32
BF16 = mybir.dt.bfloat16
I32 = mybir.dt.int32
I16 = mybir.dt.int16
AX = mybir.AxisListType
OP = mybir.AluOpType
ACT = mybir.ActivationFunctionType

N = 20000
H = 4
C = 128
IN = 512
HC = H * C
W = HC + 4            # feature cols + 4 aug (per-head separable term)
WP = 640              # DRAM row padding (dma_gather needs 256B-mult stride)
NEG = 0.2
NCORES = 8
NSH = N // NCORES     # 2500
DBLK = 125
NBLK = NSH // DBLK    # 20
ATT_EPS = 1e-10
K23 = 2.0 / 3.0


# ----------------------------------------------------------------------------
# Host-side preprocessing
# ----------------------------------------------------------------------------

def _preprocess_graph(edge_index):
    src = np.concatenate([edge_index[0], np.arange(N, dtype=np.int64)])
    dst = np.concatenate([edge_index[1], np.arange(N, dtype=np.int64)])
    deg = np.bincount(dst, minlength=N)

    nbins = NCORES * NBLK
    order = np.argsort(-deg, kind="stable")
    import heapq
    bin_load = np.zeros(nbins, np.int64)
    bin_fill = np.zeros(nbins, np.int64)
    assign = np.zeros(N, np.int64)
    heap = [(0, b) for b in range(nbins)]
    heapq.heapify(heap)
    for nid in order:
        while True:
            load, b = heapq.heappop(heap)
            if bin_fill[b] < DBLK:
                break
        assign[nid] = b
        bin_fill[b] += 1
        bin_load[b] = load + deg[nid]
        if bin_fill[b] < DBLK:
            heapq.heappush(heap, (bin_load[b], b))

    perm = np.argsort(assign * N + np.arange(N), kind="stable")
    inv_perm = np.empty(N, np.int64)
    inv_perm[perm] = np.arange(N)

    e_bin = assign[dst]
    e_dst_pos = inv_perm[dst]
    e_src_pos = inv_perm[src]
    max_per_bin = int(np.bincount(e_bin, minlength=nbins).max())
    e_blk = -(-max_per_bin // 128) * 128
    S = e_blk // 128

    order_e = np.argsort(e_bin, kind="stable")
    eb = e_bin[order_e]
    starts = np.searchsorted(eb, np.arange(nbins))
    ends = np.searchsorted(eb, np.arange(nbins), side="right")

    E16 = e_blk // 16
    src16 = np.zeros((NCORES, NBLK, 16, E16), np.int16)
    dst16 = np.zeros((NCORES, NBLK, 16, E16), np.int16)
    onehot = np.zeros((NCORES, 128, NBLK, S, DBLK), bfloat16)

    for b in range(nbins):
        core, blk = divmod(b, NBLK)
        sel = order_e[starts[b]:ends[b]]
        n = len(sel)
        pos = np.arange(n)
        src16[core, blk, pos % 16, pos // 16] = e_src_pos[sel]
        dst16[core, blk, pos % 16, pos // 16] = e_dst_pos[sel] % NSH
        onehot[core, pos % 128, blk, pos // 128, e_dst_pos[sel] % DBLK] = 1.0

    per_core = []
    for core in range(NCORES):
        # [NBLK, 16, E16] -> [16, NBLK*E16] -> replicate to 128 partitions
        s16 = src16[core].transpose(1, 0, 2).reshape(16, -1)
        d16 = dst16[core].transpose(1, 0, 2).reshape(16, -1)
        per_core.append(dict(
            src_idx=np.tile(s16, (8, 1)).copy(),
            dst_idx=np.tile(d16, (8, 1)).copy(),
            onehot=onehot[core].reshape(128, -1).copy(),
        ))
    return per_core, dict(e_blk=e_blk, perm=perm)


def _prep_weights(inputs):
    """Per layer, build augmented scaled weights.

    Column order: per head, positive-att cols then negative-att cols.
    Feature cols scaled by (2/3)*|att| (so pos-minus-neg abs-reduce gives
    (2/3)*T2); aug col 512+h = sum_c sigma_c * (|a_c| W[:,c]) gives T1.
    logit = T1 + (2/3)T2 then exp(scale=0.6).
    """
    out = {}
    npos = []
    col_perms = []
    for l in range(2):
        att = np.asarray(inputs[f"att{l}"], np.float32)
        cols = []
        np_l = []
        for h in range(H):
            pos = np.where(att[h] >= 0)[0]
            neg = np.where(att[h] < 0)[0]
            cols.append(h * C + np.concatenate([pos, neg]))
            np_l.append(len(pos))
        cols = np.concatenate(cols)
        absa = np.maximum(np.abs(att.reshape(HC)[cols]), ATT_EPS)
        sigma = np.sign(att.reshape(HC)[cols])
        sigma[sigma == 0] = 1.0
        col_perms.append(cols)
        npos.append(np_l)

        Wl = np.asarray(inputs[f"Wl{l}"], np.float32)
        Wr = np.asarray(inputs[f"Wr{l}"], np.float32)
        bl = np.asarray(inputs[f"bl{l}"], np.float32)
        br = np.asarray(inputs[f"br{l}"], np.float32)
        if l == 1:
            Wl = Wl[col_perms[0], :]
            Wr = Wr[col_perms[0], :]
        Wl = Wl[:, cols]
        Wr = Wr[:, cols]
        bl = bl[cols]
        br = br[cols]

        # scaled feature block + aug cols
        def aug(Wmat, bvec):
            Ws = Wmat * (K23 * absa)[None, :]
            bs = bvec * (K23 * absa)
            Wa = np.zeros((Wmat.shape[0], 4), np.float32)
            ba = np.zeros(4, np.float32)
            for h in range(H):
                sl = slice(h * C, (h + 1) * C)
                Wa[:, h] = (Wmat[:, sl] * (absa * sigma)[None, sl]).sum(1)
                ba[h] = (bvec[sl] * (absa * sigma)[sl]).sum()
            return (np.concatenate([Ws, Wa], 1).astype(bfloat16),
                    np.concatenate([bs, ba]).astype(np.float32))

        wl_a, bl_a = aug(Wl, bl)
        wr_a, br_a = aug(Wr, br)
        out[f"wl{l}"] = wl_a                     # [IN, 516] bf16
        out[f"wr{l}"] = wr_a
        inv = 1.0 / (K23 * absa)
        auxrows = np.zeros((4, W), np.float32)
        auxrows[0, :] = np.concatenate([bl_a[:HC] * 0 + bl_a[:HC], bl_a[HC:]])
        auxrows[0] = bl_a
        auxrows[1] = br_a
        auxrows[2, :HC] = inv
        auxrows[3, :HC] = np.asarray(inputs[f"bias{l}"], np.float32)[cols]
        out[f"aux{l}"] = auxrows
    return out, npos, col_perms


# ----------------------------------------------------------------------------
# Device kernel
# ----------------------------------------------------------------------------

def _build(e_blk, npos):
    S = e_blk // 128
    nc = bacc.Bacc("TRN2", target_bir_lowering=False, debug=False,
                   num_devices=NCORES)

    x_in = nc.dram_tensor("x_shard", [NSH, IN], F32, kind="ExternalInput")
    wl_d = [nc.dram_tensor(f"wl{l}", [IN, W], BF16, kind="ExternalInput")
            for l in range(2)]
    wr_d = [nc.dram_tensor(f"wr{l}", [IN, W], BF16, kind="ExternalInput")
            for l in range(2)]
    aux_d = [nc.dram_tensor(f"aux{l}", [4, W], F32, kind="ExternalInput")
             for l in range(2)]
    E16 = e_blk // 16
    srcidx_d = nc.dram_tensor("src_idx", [128, NBLK * E16], I16,
                              kind="ExternalInput")
    dstidx_d = nc.dram_tensor("dst_idx", [128, NBLK * E16], I16,
                              kind="ExternalInput")
    oh_d = nc.dram_tensor("onehot", [128, NBLK * S * DBLK], BF16,
                          kind="ExternalInput")
    out_d = nc.dram_tensor("out", [NSH, HC], F32, kind="ExternalOutput")

    from concourse.masks import make_identity

    with tile.TileContext(nc) as tc:
        with tc.tile_pool(name="dram", bufs=1, space="DRAM") as dram, \
             tc.tile_pool(name="const", bufs=1) as cp, \
             tc.tile_pool(name="pha", bufs=2) as wp, \
             tc.tile_pool(name="gath", bufs=2) as gp, \
             tc.tile_pool(name="sm", bufs=2) as sp, \
             tc.tile_pool(name="psum", bufs=2, space="PSUM") as pp:

            xl_sh = [dram.tile([NSH, WP], BF16, name=f"xl_sh{l}") for l in range(2)]
            xr_sh = [dram.tile([NSH, WP], BF16, name=f"xr_sh{l}") for l in range(2)]
            xl_full = [dram.tile([N, WP], BF16, name=f"xl_full{l}")
                       for l in range(2)]
            h_mid = dram.tile([NSH, HC], F32, name="h_mid")

            ident = cp.tile([DBLK, DBLK], BF16, name="ident")
            make_identity(nc, ident[:])

            si_t = cp.tile([128, NBLK * E16], I16, name="si_t")
            di_t = cp.tile([128, NBLK * E16], I16, name="di_t")
            nc.sync.dma_start(out=si_t[:], in_=srcidx_d[:])
            nc.sync.dma_start(out=di_t[:], in_=dstidx_d[:])

            for l in range(2):
                # ---- phase A ----------------------------------------------
                wl_t = cp.tile([128, 4, W], BF16, name="wl_t", tag="wl_t")
                wr_t = cp.tile([128, 4, W], BF16, name="wr_t", tag="wr_t")
                for k in range(4):
                    nc.sync.dma_start(out=wl_t[:, k, :],
                                      in_=wl_d[l][k * 128:(k + 1) * 128, :])
                    nc.sync.dma_start(out=wr_t[:, k, :],
                                      in_=wr_d[l][k * 128:(k + 1) * 128, :])
                aux_b = []
                for r in range(4):
                    row = cp.tile([1, W], F32, name=f"ar{r}", tag=f"ar{r}")
                    nc.sync.dma_start(out=row[:], in_=aux_d[l][r:r + 1, :])
                    bc = cp.tile([128, W], F32, name=f"ab{r}", tag=f"ab{r}")
                    nc.gpsimd.partition_broadcast(bc[:], row[:])
                    aux_b.append(bc)
                bl_b, br_b, invatt_b, bias_b = aux_b

                src_x = x_in if l == 0 else h_mid
                for t in range(NBLK):
                    x_t = wp.tile([DBLK, IN], BF16, name="x_t", tag="x_t")
                    nc.gpsimd.dma_start(
                        out=x_t[:], in_=src_x[t * DBLK:(t + 1) * DBLK, :])
                    xT = wp.tile([128, 4, DBLK], BF16, name="xT", tag="xT")
                    for k in range(4):
                        ps_tr = pp.tile([128, DBLK], BF16, name="ps_tr",
                                        tag="ps_a")
                        nc.tensor.transpose(out=ps_tr[:],
                                            in_=x_t[:, k * 128:(k + 1) * 128],
                                            identity=ident[:])
                        nc.scalar.copy(out=xT[:, k, :], in_=ps_tr[:])
                    ps_xl = pp.tile([DBLK, HC], F32, name="ps_xl", tag="ps_b")
                    ps_xr = pp.tile([DBLK, HC], F32, name="ps_xr", tag="ps_c")
                    ps_al = pp.tile([DBLK, 8], F32, name="ps_al",
                                    tag="ps_d", bufs=1)
                    for k in range(4):
                        nc.tensor.matmul(out=ps_xl[:], lhsT=xT[:, k, :],
                                         rhs=wl_t[:, k, 0:HC],
                                         start=(k == 0), stop=(k == 3))
                    for k in range(4):
                        nc.tensor.matmul(out=ps_xr[:], lhsT=xT[:, k, :],
                                         rhs=wr_t[:, k, 0:HC],
                                         start=(k == 0), stop=(k == 3))
                    for k in range(4):
                        nc.tensor.matmul(out=ps_al[:, 0:4], lhsT=xT[:, k, :],
                                         rhs=wl_t[:, k, HC:W],
                                         start=(k == 0), stop=(k == 3))
                    for k in range(4):
                        nc.tensor.matmul(out=ps_al[:, 4:8], lhsT=xT[:, k, :],
                                         rhs=wr_t[:, k, HC:W],
                                         start=(k == 0), stop=(k == 3))
                    xl_o = wp.tile([DBLK, W], BF16, name="xl_o", tag="xl_o")
                    xr_o = wp.tile([DBLK, W], BF16, name="xr_o", tag="xr_o")
                    nc.vector.tensor_add(out=xl_o[:, 0:HC], in0=ps_xl[:],
                                         in1=bl_b[:DBLK, 0:HC])
                    nc.vector.tensor_add(out=xr_o[:, 0:HC], in0=ps_xr[:],
                                         in1=br_b[:DBLK, 0:HC])
                    nc.vector.tensor_add(out=xl_o[:, HC:W], in0=ps_al[:, 0:4],
                                         in1=bl_b[:DBLK, HC:W])
                    nc.vector.tensor_add(out=xr_o[:, HC:W], in0=ps_al[:, 4:8],
                                         in1=br_b[:DBLK, HC:W])
                    nc.sync.dma_start(
                        out=xl_sh[l][t * DBLK:(t + 1) * DBLK, 0:W],
                        in_=xl_o[:])
                    nc.sync.dma_start(
                        out=xr_sh[l][t * DBLK:(t + 1) * DBLK, 0:W],
                        in_=xr_o[:])

                nc.gpsimd.collective_compute(
                    "AllGather", OP.bypass,
                    replica_groups=[list(range(NCORES))],
                    ins=[xl_sh[l][:]], outs=[xl_full[l][:]],
                )

                # ---- edge phase -------------------------------------------
                for b in range(NBLK):
                    oh_b = gp.tile([128, S, DBLK], BF16, name="oh_b",
                                   tag="oh_b")
                    nc.sync.dma_start(
                        out=oh_b[:],
                        in_=oh_d[:, b * S * DBLK:(b + 1) * S * DBLK])
                    xl_g = gp.tile([128, S, WP], BF16, name="xl_g",
                                   tag="xl_g")
                    xr_g = gp.tile([128, S, WP], BF16, name="xr_g",
                                   tag="xr_g")
                    for c0 in range(0, e_blk, 1024):
                        n = min(1024, e_blk - c0)
                        io0 = b * E16 + c0 // 16
                        nc.gpsimd.dma_gather(
                            out_ap=xl_g[:, c0 // 128:(c0 + n) // 128, :],
                            in_ap=xl_full[l][:],
                            idxs_ap=si_t[:, io0:io0 + n // 16],
                            num_idxs=n, num_idxs_reg=n, elem_size=WP,
                            queue_num=0)
                        nc.gpsimd.dma_gather(
                            out_ap=xr_g[:, c0 // 128:(c0 + n) // 128, :],
                            in_ap=xr_sh[l][:],
                            idxs_ap=di_t[:, io0:io0 + n // 16],
                            num_idxs=n, num_idxs_reg=n, elem_size=WP,
                            queue_num=0)

                    nc.vector.tensor_add(out=xr_g[:, :, 0:W],
                                         in0=xl_g[:, :, 0:W],
                                         in1=xr_g[:, :, 0:W])

                    # abs-reduces: lg_pn [128, 2(pn), S, 4(h)]
                    lg_pn = sp.tile([128, 2, S, H], F32, name="lg_pn",
                                    tag="lg_pn")
                    for h in range(H):
                        np_h = npos[l][h]
                        lo, mid, hi = h * C, h * C + np_h, (h + 1) * C
                        if np_h > 0:
                            nc.vector.tensor_reduce(
                                out=lg_pn[:, 0, :, h], in_=xr_g[:, :, lo:mid],
                                axis=AX.X, op=OP.add,
                                apply_absolute_value=True)
                        else:
                            nc.vector.memset(lg_pn[:, 0, :, h], 0.0)
                        if np_h < C:
                            nc.vector.tensor_reduce(
                                out=lg_pn[:, 1, :, h], in_=xr_g[:, :, mid:hi],
                                axis=AX.X, op=OP.add,
                                apply_absolute_value=True)
                        else:
                            nc.vector.memset(lg_pn[:, 1, :, h], 0.0)

                    lg = sp.tile([128, S, H], F32, name="lg", tag="lg")
                    nc.vector.tensor_tensor(out=lg[:], in0=lg_pn[:, 0, :, :],
                                            in1=lg_pn[:, 1, :, :],
                                            op=OP.subtract)
                    nc.vector.tensor_tensor(out=lg[:], in0=lg[:],
                                            in1=xr_g[:, :, HC:W],
                                            op=OP.add)

                    # xa: [128, S, 2, 258]; cols 256:258 of each hp get p
                    xa = gp.tile([128, S, 2, 258], BF16, name="xa", tag="xa")
                    nc.scalar.activation(
                        out=xa[:, :, :, 256:258],
                        in_=lg[:].rearrange("p s (a b) -> p s a b", a=2),
                        func=ACT.Exp, scale=0.6)
                    for hp in range(2):
                        nc.vector.tensor_tensor(
                            out=xa[:, :, hp, 0:256].rearrange(
                                "p s (h c) -> p s h c", h=2),
                            in0=xl_g[:, :, hp * 256:(hp + 1) * 256].rearrange(
                                "p s (h c) -> p s h c", h=2),
                            in1=xa[:, :, hp, 256:258][:, :, :, None]
                                .to_broadcast([128, S, 2, 128]),
                            op=OP.mult)

                    ps_of = [pp.tile([DBLK, HC], F32, name=f"ps_o{hp}",
                                     tag=f"ps_{'bc'[hp]}") for hp in range(2)]
                    ps_o = [t[:, 0:258] for t in ps_of]
                    for s in range(S):
                        oh_s = oh_b[:, s, :]
                        for hp in range(2):
                            nc.tensor.matmul(out=ps_o[hp][:], lhsT=oh_s,
                                             rhs=xa[:, s, hp, :],
                                             start=(s == 0), stop=(s == S - 1))

                    rinv = sp.tile([DBLK, 4], F32, name="rinv", tag="rinv")
                    for hp in range(2):
                        nc.vector.reciprocal(out=rinv[:, 2 * hp:2 * hp + 2],
                                             in_=ps_o[hp][:, 256:258])
                    o_sb = sp.tile([DBLK, HC], F32, name="o_sb", tag="o_sb")
                    for hp in range(2):
                        nc.vector.tensor_tensor(
                            out=o_sb[:, hp * 256:(hp + 1) * 256].rearrange(
                                "p (h c) -> p h c", h=2),
                            in0=ps_o[hp][:, 0:256].rearrange(
                                "p (h c) -> p h c", h=2),
                            in1=rinv[:, 2 * hp:2 * hp + 2][:, :, None]
                                .to_broadcast([DBLK, 2, 128]),
                            op=OP.mult)
                    nc.vector.tensor_mul(out=o_sb[:], in0=o_sb[:],
                                         in1=invatt_b[:DBLK, 0:HC])
                    nc.vector.tensor_add(out=o_sb[:], in0=o_sb[:],
                                         in1=bias_b[:DBLK, 0:HC])
                    rows = slice(b * DBLK, (b + 1) * DBLK)
                    if l == 0:
                        r_t = sp.tile([DBLK, HC], F32, name="r_t", tag="r_t")
                        nc.scalar.activation(out=r_t[:], in_=o_sb[:],
                                             func=ACT.Relu)
                        e_t = sp.tile([DBLK, HC], F32, name="e_t", tag="e_t")
                        nc.scalar.activation(out=e_t[:], in_=o_sb[:],
                                             func=ACT.Exp)
                        nc.vector.tensor_scalar(
                            out=e_t[:], in0=e_t[:], scalar1=-1.0, scalar2=0.0,
                            op0=OP.add, op1=OP.min)
                        nc.vector.tensor_add(out=r_t[:], in0=r_t[:],
                                             in1=e_t[:])
                        nc.sync.dma_start(out=h_mid[rows, :], in_=r_t[:])
                    else:
                        nc.sync.dma_start(out=out_d[rows, :], in_=o_sb[:])

    nc.compile()
    return nc


_CACHE = {}


def _get_nc(e_blk, npos_key):
    key = (e_blk, npos_key)
    if key not in _CACHE:
        _CACHE[key] = _build(e_blk, [list(npos_key[0]), list(npos_key[1])])
    return _CACHE[key]


def kernel(**inputs):
    per_core, meta = _preprocess_graph(np.asarray(inputs["edge_index"]))
    wprep, npos, col_perms = _prep_weights(inputs)
    e_blk = meta["e_blk"]
    perm = meta["perm"]

    nc = _get_nc(e_blk, (tuple(npos[0]), tuple(npos[1])))

    x = np.asarray(inputs["x"], np.float32)
    x_perm = x[perm]
    in_maps = []
    for core in range(NCORES):
        m = dict(
            x_shard=np.ascontiguousarray(x_perm[core * NSH:(core + 1) * NSH]),
            src_idx=per_core[core]["src_idx"],
            dst_idx=per_core[core]["dst_idx"],
            onehot=per_core[core]["onehot"],
        )
        for l in range(2):
            m[f"wl{l}"] = wprep[f"wl{l}"]
            m[f"wr{l}"] = wprep[f"wr{l}"]
            m[f"aux{l}"] = wprep[f"aux{l}"]
        in_maps.append(m)

    trace = bool(inputs.pop("_trace", False))
    res = run_bass_kernel_spmd(nc, in_maps, core_ids=list(range(NCORES)),
                               trace=trace)
    out_rows = np.concatenate([res.results[c]["out"] for c in range(NCORES)],
                              axis=0)
    out = np.zeros((N, HC), np.float32)
    tmp = np.zeros((N, HC), np.float32)
    tmp[perm] = out_rows
    out[:, col_perms[1]] = tmp
    if trace:
        kernel._last_result = res
    return out
